# revision 1
# baseline (speedup 1.0000x reference)
"""GTN (graph transformer network) meta-path kernel for TRN2, 8 NeuronCores.

Math (reference):
    Ap = A transposed to [E, N, N]
    a  = sum_e softmax(w1_0)[c,e] * Ap[e]      (per channel c)
    b  = sum_e softmax(w2_0)[c,e] * Ap[e]
    H  = a @ b
    twice:  H = normalize(H) @ gtconv(Ap, w)   (normalize = zero diag, col-scale)
    out = symmetrized mean over channels.

Sharding: channel-parallel — core c computes channel c end to end (the four
softmax mixes differ only in the tiny [E] weight vector, fed per-core), then
one AllReduce over the 8 cores and a local symmetrization.

On-device formulation works with TRANSPOSED intermediates Ht = H^T so that
 - every GEMM's moving operand is the previous GEMM's output as-is,
 - normalization becomes row sums (free-dim reduce) + per-partition scale.

The edge-type mix contracts e on SBUF partitions, so the host feeds A once
in a pre-permuted bf16 layout At3[b, (k16 e), j]; one PE pass with a
block-diagonal [128, 64] weight computes all four mixes in natural
orientation, written to DRAM in the PSUM-packed row order (row = kb*64 +
q*16 + k16).  GEMM stationaries load straight from that packed layout with
multi-dim APs; the 'a' mix is unpacked once (DRAM->DRAM) and transposed on
the PE into the GEMM1 moving operand.  HWDGE DMA issue costs ~0.6 us of
sequencer time each, so the design minimizes DMA instruction count.
"""

import numpy as np

N = 2048
E = 8
C = 8
P = 128
NCORES = 8

_PROGRAM = None


def _softmax_rows(w: np.ndarray) -> np.ndarray:
    """w: [C, E, 1, 1] -> softmax over E, float64 precision, returns [C, E]."""
    x = w.reshape(C, E).astype(np.float64)
    x = x - x.max(axis=1, keepdims=True)
    ex = np.exp(x)
    return ex / ex.sum(axis=1, keepdims=True)


def _build_program():
    import concourse.bacc as bacc
    import concourse.mybir as mybir
    import concourse.tile as tile
    from concourse.masks import make_identity

    f32 = mybir.dt.float32
    bf16 = mybir.dt.bfloat16
    AX = mybir.AxisListType.X
    MUL = mybir.AluOpType.mult
    ADD = mybir.AluOpType.add
    NE = mybir.AluOpType.not_equal
    COPY = mybir.ActivationFunctionType.Copy

    nc = bacc.Bacc("TRN2")
    A3_ext = nc.dram_tensor("At3", [P, P, N], bf16, kind="ExternalInput")
    w4_ext = nc.dram_tensor("wblk4", [P, 64], bf16, kind="ExternalInput")
    out_ext = nc.dram_tensor("out", [N, N], f32, kind="ExternalOutput")

    with tile.TileContext(nc) as tc:
        with (
            tc.tile_pool(name="dram", bufs=1, space="DRAM") as dpool,
            tc.tile_pool(name="const", bufs=1) as cpool,
        ):
            # all four mixes, psum-packed: row = kb*64 + q*16 + k16
            # quartered so unpacking can start before the whole mix finishes
            packed = [
                dpool.tile([N, N], bf16, name=f"packed{qt}") for qt in range(4)
            ]
            anat = dpool.tile([N, N], bf16)         # a in natural [i, kappa]
            nat = [dpool.tile([N, N], bf16, name=f"nat{q}") for q in range(1, 4)]
            # per-channel H''^T and allreduced sum, in 4 row bands so
            # collectives pipeline with GEMM3 and phase 6
            h2t = [dpool.tile([512, N], f32, name=f"h2t{b}") for b in range(4)]
            s_sh = [
                dpool.tile([512, N], f32, addr_space="Shared", name=f"ssh{b}")
                for b in range(4)
            ]

            # --- constants ---
            w4_sb = cpool.tile([P, 64], bf16)
            nc.sync.dma_start(out=w4_sb[:], in_=w4_ext[:])
            ident = cpool.tile([P, P], f32)
            make_identity(nc, ident[:])
            identb = cpool.tile([P, P], bf16)
            make_identity(nc, identb[:])
            # diag masks: masks[:, v, y] = 0 where y == p + v*128 else 1
            masks = cpool.tile([P, 4, 512], f32)
            nc.gpsimd.memset(masks[:], 1.0)
            for v in range(4):
                nc.gpsimd.affine_select(
                    out=masks[:, v],
                    in_=masks[:, v],
                    compare_op=NE,
                    fill=0.0,
                    base=v * P,
                    pattern=[[-1, 512]],
                    channel_multiplier=1,
                )

            # =========== Phase 1: all four mixes in one PE pass ===========
            with (
                tc.tile_pool(name="mix", bufs=3) as mpool,
                tc.tile_pool(name="mixst", bufs=8) as spool,
                tc.tile_pool(name="mixps", bufs=6, space="PSUM") as mpsum,
            ):
                for ld4 in range(32):
                    a3t = mpool.tile([P, 4, N], bf16, tag="a3t")
                    nc.sync.dma_start(
                        out=a3t[:],
                        in_=A3_ext[4 * ld4 : 4 * ld4 + 4].rearrange(
                            "b p j -> p b j"
                        ),
                    )
                    for half in range(2):
                        bp = ld4 * 2 + half
                        qt, bpl = bp // 16, bp % 16
                        for jc in range(4):
                            pm = mpsum.tile([P, 512], f32, tag="pm")
                            for h in range(2):
                                nc.tensor.matmul(
                                    pm[h * 64 : (h + 1) * 64, :],
                                    lhsT=w4_sb[:],
                                    rhs=a3t[
                                        :,
                                        half * 2 + h,
                                        jc * 512 : (jc + 1) * 512,
                                    ],
                                    start=True,
                                    stop=True,
                                )
                            st = spool.tile([P, 512], bf16, tag="st")
                            if jc % 2 == 0:
                                nc.vector.tensor_copy(out=st[:], in_=pm[:])
                            else:
                                nc.scalar.copy(st[:], pm[:])
                            weng = nc.scalar if jc % 2 == 0 else nc.sync
                            weng.dma_start(
                                out=packed[qt][
                                    bpl * P : (bpl + 1) * P,
                                    jc * 512 : (jc + 1) * 512,
                                ],
                                in_=st[:],
                            )
                    if ld4 % 8 == 7:
                        # this quarter of packed is complete: unpack (d2d)
                        qt = ld4 // 8
                        pk5 = packed[qt][:].rearrange(
                            "(bp h q k) j -> bp h q k j", h=2, q=4, k=16
                        )
                        for q in range(2):
                            dst_plane = anat if q == 0 else nat[q - 1]
                            d5 = dst_plane[:].rearrange(
                                "(qt bp h k) j -> qt bp h k j",
                                qt=4, h=2, k=16,
                            )
                            for h in range(2):
                                eng = nc.sync if h == 0 else nc.scalar
                                eng.dma_start(
                                    out=d5[qt, :, h], in_=pk5[:, h, q]
                                )

            # =========== Phases 2-4: three chained GEMMs ===========
            with (
                tc.tile_pool(name="big", bufs=1) as bigpool,
                tc.tile_pool(name="gw", bufs=3) as gpool,
                tc.tile_pool(name="nrm", bufs=4) as npool,
                tc.tile_pool(name="gps", bufs=2, space="PSUM") as gpsum,
            ):
                mv = [
                    bigpool.tile([P, 16, N], bf16, tag="mv0", name="mva"),
                    bigpool.tile([P, 16, N], bf16, tag="mv1", name="mvb"),
                ]

                # Build mv0 = a^T chunks by PE-transposing anat blocks
                anat_v = anat[:].rearrange("(ib p) k -> p ib k", p=P)
                for kc in range(16):
                    ld = gpool.tile([P, 16, P], bf16, tag="ld")
                    nc.sync.dma_start(
                        out=ld[:], in_=anat_v[:, :, kc * P : (kc + 1) * P]
                    )
                    for ib4 in range(4):
                        tp = gpsum.tile(
                            [P, 512], bf16, tag=f"ps{ib4 % 2}", name="tp"
                        )
                        for g in range(4):
                            nc.tensor.transpose(
                                tp[:, g * P : (g + 1) * P],
                                ld[:, ib4 * 4 + g, :],
                                identb[:],
                            )
                        if ib4 % 2 == 0:
                            nc.vector.tensor_copy(
                                out=mv[0][:, kc, ib4 * 512 : (ib4 + 1) * 512],
                                in_=tp[:],
                            )
                        else:
                            nc.scalar.copy(
                                mv[0][:, kc, ib4 * 512 : (ib4 + 1) * 512],
                                tp[:],
                            )

                def gemm(qi, rhs_res, out_res, normalize):
                    """Transposed-chain GEMM: out = mix_q^T @ rhs.

                    qi: q index in packed (1=b, 2=g1, 3=g2).
                    rhs_res: SBUF-resident moving operand [P, 16, N] bf16.
                    out_res: SBUF [P, 16, N] bf16 (normalize) or None (evict
                        f32 to h2t).
                    """
                    for ms in range(16):
                        bts = gpool.tile([P, 16, P], bf16, tag="bts")
                        nc.sync.dma_start(
                            out=bts[:],
                            in_=nat[qi - 1][:].rearrange(
                                "(kc p) j -> p kc j", p=P
                            )[:, :, ms * P : (ms + 1) * P],
                        )
                        ps = [
                            gpsum.tile(
                                [P, 512], f32, tag=f"ps{ic}", name=f"ps{ic}"
                            )
                            for ic in range(4)
                        ]
                        for kc in range(16):
                            for ic in range(4):
                                nc.tensor.matmul(
                                    ps[ic][:],
                                    lhsT=bts[:, kc, :],
                                    rhs=rhs_res[:, kc, ic * 512 : (ic + 1) * 512],
                                    start=(kc == 0),
                                    stop=(kc == 15),
                                )
                        if normalize:
                            dc = (ms * P) // 512
                            v = ms % 4
                            degp = npool.tile([P, 4], f32, tag="degp")
                            # zero the diagonal in place + row-sum of masked tile
                            nc.vector.scalar_tensor_tensor(
                                out=ps[dc][:],
                                in0=ps[dc][:],
                                scalar=1.0,
                                in1=masks[:, v],
                                op0=MUL,
                                op1=MUL,
                                accum_out=degp[:, dc : dc + 1],
                            )
                            for ic in range(4):
                                if ic != dc:
                                    nc.vector.tensor_reduce(
                                        degp[:, ic : ic + 1], ps[ic][:], AX, ADD
                                    )
                            degs = npool.tile([P, 1], f32, tag="degs")
                            nc.vector.tensor_reduce(degs[:], degp[:], AX, ADD)
                            dinv = npool.tile([P, 1], f32, tag="dinv")
                            nc.vector.reciprocal(dinv[:], degs[:])
                            for ic in range(4):
                                nc.scalar.activation(
                                    out_res[:, ms, ic * 512 : (ic + 1) * 512],
                                    ps[ic][:],
                                    COPY,
                                    scale=dinv[:],
                                )
                        else:
                            for ic in range(4):
                                st = gpool.tile([P, 512], f32, tag="fstage")
                                nc.scalar.copy(st[:], ps[ic][:])
                                nc.scalar.dma_start(
                                    out=h2t[ms // 4][
                                        (ms % 4) * P : (ms % 4 + 1) * P,
                                        ic * 512 : (ic + 1) * 512,
                                    ],
                                    in_=st[:],
                                )

                # GEMM1: Ht = b^T a^T ; normalize -> Hnt in mv[1]
                gemm(1, mv[0], mv[1], normalize=True)

                # unpack g1/g2 now - overlaps GEMM1/2 compute (HBM is idle)
                for q in range(2, 4):
                    d5 = nat[q - 1][:].rearrange(
                        "(qt bp h k) j -> qt bp h k j", qt=4, h=2, k=16
                    )
                    for qt in range(4):
                        pk5l = packed[qt][:].rearrange(
                            "(bp h q k) j -> bp h q k j", h=2, q=4, k=16
                        )
                        for h in range(2):
                            nc.gpsimd.dma_start(
                                out=d5[qt, :, h], in_=pk5l[:, h, q]
                            )
                # GEMM2: H't = g1^T Hnt ; normalize -> H'nt (reuse mv0 slot)
                mv0b = bigpool.tile([P, 16, N], bf16, tag="mv0")
                gemm(2, mv[1], mv0b, normalize=True)
                # GEMM3: H''t = g2^T H'nt -> h2t (f32), g2 pre-scaled by 1/16
                gemm(3, mv0b, None, normalize=False)

                # ===== Phase 5: banded AllReduce, pipelined with GEMM3 =====
                for b in range(4):
                    nc.gpsimd.collective_compute(
                        "AllReduce",
                        ADD,
                        replica_groups=[list(range(NCORES))],
                        ins=[h2t[b].opt()],
                        outs=[s_sh[b].opt()],
                    )

                # ===== Phase 6: symmetrize out = S + S^T (banded) =====
                s_cols = [
                    s_sh[b][:].rearrange("(nb p) m -> p nb m", p=P)
                    for b in range(4)
                ]
                for ms in range(16):
                    srow = gpool.tile([P, N], f32, tag="srow", bufs=2)
                    nc.sync.dma_start(
                        out=srow[:],
                        in_=s_sh[ms // 4][(ms % 4) * P : (ms % 4 + 1) * P, :],
                    )
                    ost = gpool.tile([P, N], f32, tag="ost", bufs=2)
                    for b in range(4):
                        colb = gpool.tile([P, 4, P], f32, tag="colb")
                        nc.sync.dma_start(
                            out=colb[:],
                            in_=s_cols[b][:, :, ms * P : (ms + 1) * P],
                        )
                        pst = gpsum.tile(
                            [P, 512], f32, tag=f"ps{b % 2}", name="pst"
                        )
                        for g in range(4):
                            nc.tensor.transpose(
                                pst[:, g * P : (g + 1) * P],
                                colb[:, g, :],
                                ident[:],
                            )
                        nc.vector.scalar_tensor_tensor(
                            out=ost[:, b * 512 : (b + 1) * 512],
                            in0=srow[:, b * 512 : (b + 1) * 512],
                            scalar=1.0,
                            in1=pst[:],
                            op0=MUL,
                            op1=ADD,
                        )
                    nc.scalar.dma_start(
                        out=out_ext[ms * P : (ms + 1) * P, :], in_=ost[:]
                    )


    nc.compile()
    return nc


def _get_program():
    global _PROGRAM
    if _PROGRAM is None:
        _PROGRAM = _build_program()
    return _PROGRAM


def _make_wblk(sws) -> np.ndarray:
    """Block-diagonal mix weights [128, 16*len(sws)].

    wblk[(x*8+e), (q*16+x)] = sws[q][e]  for x in 0..15.
    Partitions = (16 x, 8 e) matching the host-permuted A layout; out
    partitions = (q, 16 x).
    """
    wblk = np.zeros((P, 16 * len(sws)), np.float32)
    for q, sw in enumerate(sws):
        for x in range(16):
            wblk[x * 8 : (x + 1) * 8, q * 16 + x] = sw.astype(np.float32)
    return wblk


def _prep_inputs(A, w1_0, w2_0, w_1, w_2):
    import ml_dtypes

    swa = _softmax_rows(np.asarray(w1_0))
    swb = _softmax_rows(np.asarray(w2_0))
    sg1 = _softmax_rows(np.asarray(w_1))
    # fold mean over channels (1/8) and symmetrize (1/2) into the last mix
    sg2 = _softmax_rows(np.asarray(w_2)) / 16.0

    abf = np.asarray(A, dtype=np.float32)[0].astype(ml_dtypes.bfloat16)  # [k,j,e]
    # At3[b, (k16 e), j] = A[16b+k16, j, e]
    at3 = np.ascontiguousarray(abf.transpose(0, 2, 1).reshape(P, P, N))
    in_maps = []
    for c in range(NCORES):
        w4 = _make_wblk([swa[c], swb[c], sg1[c], sg2[c]]).astype(
            ml_dtypes.bfloat16
        )
        in_maps.append({"At3": at3, "wblk4": w4})
    return in_maps


def kernel(A, w1_0, w2_0, w_1, w_2):
    from concourse.bass_utils import run_bass_kernel_spmd

    in_maps = _prep_inputs(A, w1_0, w2_0, w_1, w_2)
    nc = _get_program()
    res = run_bass_kernel_spmd(nc, in_maps, list(range(NCORES)))
    return np.asarray(res.results[0]["out"], dtype=np.float32)



# revision 4
# speedup vs baseline: 1.6189x; 1.6189x over previous
"""GTN (graph transformer network) meta-path kernel for TRN2, 8 NeuronCores.

Math (reference):
    Ap = A transposed to [E, N, N]
    a  = sum_e softmax(w1_0)[c,e] * Ap[e]      (per channel c)
    b  = sum_e softmax(w2_0)[c,e] * Ap[e]
    H  = a @ b
    twice:  H = normalize(H) @ gtconv(Ap, w)   (normalize = zero diag, col-scale)
    out = symmetrized mean over channels.

Sharding: channel-parallel — core c computes channel c end to end (the four
softmax mixes differ only in the tiny [E] weight vector, fed per-core), then
one AllReduce over the 8 cores and a local symmetrization.

On-device formulation works with TRANSPOSED intermediates Ht = H^T so that
 - every GEMM's moving operand is the previous GEMM's output as-is,
 - normalization becomes row sums (free-dim reduce) + per-partition scale.

v2 design notes (vs the packed/unpack baseline):
 - A is fed once in a host-permuted bf16 layout At3[b, (k16 e), j]; one PE
   pass with a block-diagonal [128, 64] weight computes all four mixes.
 - Mix outputs are written DIRECTLY to a natural-layout fp8 tensor nat4
   [4, N, N] (no DRAM packed round-trip): full-row staging in SBUF gives
   per-h scatter DMAs with 3-dim APs (q, k, j) and 2KB contiguous lines.
 - A-loads round-robin across the three DMA-capable queues (sync / scalar /
   gpsimd) — a single queue tops out ~130-185 GB/s and serialized the
   baseline's mix phase.
 - GEMMs run in fp8 (e4m3) with DoubleRow perf mode (contract 256/instr).
   Normalized intermediates are scaled by N so fp8 sees ~1.0-magnitudes
   (normalize is scale-invariant, so the chain stays exact); the single
   1/(N*16) correction is folded into the GEMM3 PSUM eviction.
 - h2t / AllReduce in bf16 (halves collective + tail HBM traffic).
"""

import numpy as np

N = 2048
E = 8
C = 8
P = 128
NCORES = 8
SCALE_N = float(N)          # fp8 re-scale applied at each normalize
EVICT_SCALE = 1.0 / (SCALE_N * 16.0)  # undo SCALE_N; /8 channel mean; /2 symm

_PROGRAM = None


def _softmax_rows(w: np.ndarray) -> np.ndarray:
    """w: [C, E, 1, 1] -> softmax over E, float64 precision, returns [C, E]."""
    x = w.reshape(C, E).astype(np.float64)
    x = x - x.max(axis=1, keepdims=True)
    ex = np.exp(x)
    return ex / ex.sum(axis=1, keepdims=True)


def _build_program():
    import concourse.bacc as bacc
    import concourse.mybir as mybir
    import concourse.tile as tile
    from concourse.masks import make_identity

    f32 = mybir.dt.float32
    bf16 = mybir.dt.bfloat16
    f8 = mybir.dt.float8e4
    AX = mybir.AxisListType.X
    MUL = mybir.AluOpType.mult
    ADD = mybir.AluOpType.add
    NE = mybir.AluOpType.not_equal
    COPY = mybir.ActivationFunctionType.Copy
    DR = mybir.MatmulPerfMode.DoubleRow

    nc = bacc.Bacc("TRN2")
    A3_ext = nc.dram_tensor("At3", [P, P, N], bf16, kind="ExternalInput")
    w4_ext = nc.dram_tensor("wblk4", [P, 64], bf16, kind="ExternalInput")
    out_ext = nc.dram_tensor("out", [N, N], f32, kind="ExternalOutput")

    with tile.TileContext(nc) as tc:
        with (
            tc.tile_pool(name="dram", bufs=1, space="DRAM") as dpool,
            tc.tile_pool(name="const", bufs=1) as cpool,
        ):
            # all four mixes in natural layout, fp8: nat4[q, i, j]
            nat4 = dpool.tile([4, N, N], f8, name="nat4")
            # per-channel H''^T and allreduced sum, bf16, in 4 row bands so
            # collectives pipeline with GEMM3 and phase 6
            h2t = [dpool.tile([512, N], bf16, name=f"h2t{b}") for b in range(4)]
            s_sh = [
                dpool.tile([512, N], bf16, addr_space="Shared", name=f"ssh{b}")
                for b in range(4)
            ]

            # --- constants ---
            w4_sb = cpool.tile([P, 64], bf16)
            nc.sync.dma_start(out=w4_sb[:], in_=w4_ext[:])
            identb = cpool.tile([P, P], bf16)
            make_identity(nc, identb[:])
            identf8 = cpool.tile([P, P], f8)
            make_identity(nc, identf8[:])
            # diag masks: masks[:, v, y] = 0 where y == p + v*128 else 1
            masks = cpool.tile([P, 4, 512], f32)
            nc.gpsimd.memset(masks[:], 1.0)
            for v in range(4):
                nc.gpsimd.affine_select(
                    out=masks[:, v],
                    in_=masks[:, v],
                    compare_op=NE,
                    fill=0.0,
                    base=v * P,
                    pattern=[[-1, 512]],
                    channel_multiplier=1,
                )

            # natural-layout scatter view of nat4 for the mix writes:
            # natural row i = 32*bp + 16*h + k ; partition p of a staged mix
            # tile is (h, q, k) -> per-h destination AP dims (q, k, j): 3-dim.
            nat_sc = nat4[:].rearrange(
                "q (bp h k) j -> bp h q k j", h=2, k=16
            )

            # =========== Phase 1: all four mixes in one PE pass ===========
            ld_engs = [nc.sync, nc.scalar, nc.gpsimd]
            with (
                tc.tile_pool(name="mix", bufs=3) as mpool,
                tc.tile_pool(name="mixst", bufs=3) as spool,
                tc.tile_pool(name="mixps", bufs=6, space="PSUM") as mpsum,
            ):
                for ld4 in range(32):
                    a3t = mpool.tile([P, 4, N], bf16, tag="a3t")
                    ld_engs[ld4 % 3].dma_start(
                        out=a3t[:],
                        in_=A3_ext[4 * ld4 : 4 * ld4 + 4].rearrange(
                            "b p j -> p b j"
                        ),
                    )
                    for half in range(2):
                        bp = ld4 * 2 + half
                        # full-row staging: st holds both h's for this bp
                        st = spool.tile([P, N], f8, tag="st")
                        for jc in range(4):
                            pm = mpsum.tile([P, 512], f32, tag="pm")
                            for h in range(2):
                                nc.tensor.matmul(
                                    pm[h * 64 : (h + 1) * 64, :],
                                    lhsT=w4_sb[:],
                                    rhs=a3t[
                                        :,
                                        half * 2 + h,
                                        jc * 512 : (jc + 1) * 512,
                                    ],
                                    start=True,
                                    stop=True,
                                )
                            if jc % 2 == 0:
                                nc.vector.tensor_copy(
                                    out=st[:, jc * 512 : (jc + 1) * 512],
                                    in_=pm[:],
                                )
                            else:
                                nc.scalar.copy(
                                    st[:, jc * 512 : (jc + 1) * 512], pm[:]
                                )
                        # one scatter DMA per h: dest dims (q, k, j=2048)
                        nc.sync.dma_start(
                            out=nat_sc[bp, 0], in_=st[0:64, :]
                        )
                        nc.scalar.dma_start(
                            out=nat_sc[bp, 1], in_=st[64:128, :]
                        )

            # =========== Phases 2-4: three chained GEMMs (fp8) ===========
            with (
                tc.tile_pool(name="big", bufs=1) as bigpool,
                tc.tile_pool(name="gw", bufs=3) as gpool,
                tc.tile_pool(name="nrm", bufs=4) as npool,
                tc.tile_pool(name="gps", bufs=2, space="PSUM") as gpsum,
            ):
                mv = [
                    bigpool.tile([P, 16, N], f8, tag="mv0", name="mva"),
                    bigpool.tile([P, 16, N], f8, tag="mv1", name="mvb"),
                ]

                # Build mv0 = a^T chunks by PE-transposing nat4[0] blocks
                anat_v = nat4[0].rearrange("(ib p) k -> p ib k", p=P)
                for kc in range(16):
                    ld = gpool.tile([P, 16, P], f8, tag="ld")
                    nc.sync.dma_start(
                        out=ld[:], in_=anat_v[:, :, kc * P : (kc + 1) * P]
                    )
                    for ib4 in range(4):
                        # fp8 transpose mode requires output element step 2:
                        # allocate 2x-wide psum and write via stride-2 view
                        tp = gpsum.tile(
                            [P, 1024], f8, tag=f"ps{ib4 % 2}", name="tp"
                        )
                        tpv = tp[:].rearrange("p (x two) -> p two x", two=2)[
                            :, 0
                        ]
                        for g in range(4):
                            nc.tensor.transpose(
                                tpv[:, g * P : (g + 1) * P],
                                ld[:, ib4 * 4 + g, :],
                                identf8[:],
                            )
                        if ib4 % 2 == 0:
                            nc.vector.tensor_copy(
                                out=mv[0][:, kc, ib4 * 512 : (ib4 + 1) * 512],
                                in_=tpv[:],
                            )
                        else:
                            nc.scalar.copy(
                                mv[0][:, kc, ib4 * 512 : (ib4 + 1) * 512],
                                tpv[:],
                            )

                def gemm(qi, rhs_res, out_res, normalize):
                    """Transposed-chain GEMM: out = mix_q^T @ rhs (DoubleRow).

                    qi: q index in nat4 (1=b, 2=g1, 3=g2).
                    rhs_res: SBUF-resident moving operand [P, 16, N] fp8.
                    out_res: SBUF [P, 16, N] fp8 (normalize, scaled by N) or
                        None (evict bf16 to h2t with EVICT_SCALE).
                    """
                    natq = nat4[qi].rearrange("(kc p) j -> p kc j", p=P)
                    for ms2 in range(8):
                        # stationary for two ms blocks: 512B lines
                        bts = gpool.tile([P, 16, 2 * P], f8, tag="bts")
                        nc.sync.dma_start(
                            out=bts[:],
                            in_=natq[:, :, ms2 * 2 * P : (ms2 + 1) * 2 * P],
                        )
                        for msh in range(2):
                            ms = ms2 * 2 + msh
                            ps = [
                                gpsum.tile(
                                    [P, 512], f32, tag=f"ps{ic}", name=f"ps{ic}"
                                )
                                for ic in range(4)
                            ]
                            for kc2 in range(8):
                                for ic in range(4):
                                    nc.tensor.matmul(
                                        ps[ic][:],
                                        lhsT=bts[
                                            :,
                                            2 * kc2 : 2 * kc2 + 2,
                                            msh * P : (msh + 1) * P,
                                        ],
                                        rhs=rhs_res[
                                            :,
                                            2 * kc2 : 2 * kc2 + 2,
                                            ic * 512 : (ic + 1) * 512,
                                        ],
                                        start=(kc2 == 0),
                                        stop=(kc2 == 7),
                                        perf_mode=DR,
                                    )
                            if normalize:
                                dc = (ms * P) // 512
                                v = ms % 4
                                degp = npool.tile([P, 4], f32, tag="degp")
                                # zero diagonal in place + row-sum of masked tile
                                nc.vector.scalar_tensor_tensor(
                                    out=ps[dc][:],
                                    in0=ps[dc][:],
                                    scalar=1.0,
                                    in1=masks[:, v],
                                    op0=MUL,
                                    op1=MUL,
                                    accum_out=degp[:, dc : dc + 1],
                                )
                                for ic in range(4):
                                    if ic != dc:
                                        nc.vector.tensor_reduce(
                                            degp[:, ic : ic + 1], ps[ic][:],
                                            AX, ADD,
                                        )
                                degs = npool.tile([P, 1], f32, tag="degs")
                                nc.vector.tensor_reduce(degs[:], degp[:], AX, ADD)
                                dinv = npool.tile([P, 1], f32, tag="dinv")
                                nc.vector.reciprocal(dinv[:], degs[:])
                                # fp8 range: scale normalized rows by N (~1.0)
                                nc.vector.tensor_scalar_mul(
                                    dinv[:], dinv[:], SCALE_N
                                )
                                for ic in range(4):
                                    nc.scalar.activation(
                                        out_res[:, ms, ic * 512 : (ic + 1) * 512],
                                        ps[ic][:],
                                        COPY,
                                        scale=dinv[:],
                                    )
                            else:
                                for ic in range(4):
                                    st = gpool.tile([P, 512], bf16, tag="fstage")
                                    nc.scalar.activation(
                                        st[:], ps[ic][:], COPY, scale=EVICT_SCALE
                                    )
                                    nc.scalar.dma_start(
                                        out=h2t[ms // 4][
                                            (ms % 4) * P : (ms % 4 + 1) * P,
                                            ic * 512 : (ic + 1) * 512,
                                        ],
                                        in_=st[:],
                                    )

                # GEMM1: Ht = b^T a^T ; normalize -> N*Hnt in mv[1]
                gemm(1, mv[0], mv[1], normalize=True)
                # GEMM2: H't = g1^T Hnt ; normalize -> N*H'nt (reuse mv0 slot)
                mv0b = bigpool.tile([P, 16, N], f8, tag="mv0")
                gemm(2, mv[1], mv0b, normalize=True)
                # GEMM3: H''t = g2^T H'nt -> h2t (bf16, EVICT_SCALE folded)
                gemm(3, mv0b, None, normalize=False)

                # ===== Phase 5: banded AllReduce, pipelined with GEMM3 =====
                for b in range(4):
                    nc.gpsimd.collective_compute(
                        "AllReduce",
                        ADD,
                        replica_groups=[list(range(NCORES))],
                        ins=[h2t[b].opt()],
                        outs=[s_sh[b].opt()],
                    )

                # ===== Phase 6: symmetrize out = S + S^T (banded) =====
                s_cols = [
                    s_sh[b][:].rearrange("(nb p) m -> p nb m", p=P)
                    for b in range(4)
                ]
                p6_eng = [nc.sync, nc.scalar, nc.gpsimd]
                for ms in range(16):
                    srow = gpool.tile([P, N], bf16, tag="srow", bufs=2)
                    p6_eng[ms % 3].dma_start(
                        out=srow[:],
                        in_=s_sh[ms // 4][(ms % 4) * P : (ms % 4 + 1) * P, :],
                    )
                    ost = gpool.tile([P, N], f32, tag="ost", bufs=2)
                    for b in range(4):
                        colb = gpool.tile([P, 4, P], bf16, tag="colb")
                        p6_eng[(ms + b + 1) % 3].dma_start(
                            out=colb[:],
                            in_=s_cols[b][:, :, ms * P : (ms + 1) * P],
                        )
                        pst = gpsum.tile(
                            [P, 512], bf16, tag=f"ps{b % 2}", name="pst"
                        )
                        for g in range(4):
                            nc.tensor.transpose(
                                pst[:, g * P : (g + 1) * P],
                                colb[:, g, :],
                                identb[:],
                            )
                        nc.vector.scalar_tensor_tensor(
                            out=ost[:, b * 512 : (b + 1) * 512],
                            in0=srow[:, b * 512 : (b + 1) * 512],
                            scalar=1.0,
                            in1=pst[:],
                            op0=MUL,
                            op1=ADD,
                        )
                    p6_eng[(ms + 2) % 3].dma_start(
                        out=out_ext[ms * P : (ms + 1) * P, :], in_=ost[:]
                    )


    nc.compile()
    return nc


def _get_program():
    global _PROGRAM
    if _PROGRAM is None:
        _PROGRAM = _build_program()
    return _PROGRAM


def _make_wblk(sws) -> np.ndarray:
    """Block-diagonal mix weights [128, 16*len(sws)].

    wblk[(x*8+e), (q*16+x)] = sws[q][e]  for x in 0..15.
    Partitions = (16 x, 8 e) matching the host-permuted A layout; out
    partitions = (q, 16 x).
    """
    wblk = np.zeros((P, 16 * len(sws)), np.float32)
    for q, sw in enumerate(sws):
        for x in range(16):
            wblk[x * 8 : (x + 1) * 8, q * 16 + x] = sw.astype(np.float32)
    return wblk


def _prep_inputs(A, w1_0, w2_0, w_1, w_2):
    import ml_dtypes

    swa = _softmax_rows(np.asarray(w1_0))
    swb = _softmax_rows(np.asarray(w2_0))
    sg1 = _softmax_rows(np.asarray(w_1))
    # channel-mean/symmetrize scale folded into GEMM3's PSUM eviction
    sg2 = _softmax_rows(np.asarray(w_2))

    abf = np.asarray(A, dtype=np.float32)[0].astype(ml_dtypes.bfloat16)  # [k,j,e]
    # At3[b, (k16 e), j] = A[16b+k16, j, e]
    at3 = np.ascontiguousarray(abf.transpose(0, 2, 1).reshape(P, P, N))
    in_maps = []
    for c in range(NCORES):
        w4 = _make_wblk([swa[c], swb[c], sg1[c], sg2[c]]).astype(
            ml_dtypes.bfloat16
        )
        in_maps.append({"At3": at3, "wblk4": w4})
    return in_maps


def kernel(A, w1_0, w2_0, w_1, w_2):
    from concourse.bass_utils import run_bass_kernel_spmd

    in_maps = _prep_inputs(A, w1_0, w2_0, w_1, w_2)
    nc = _get_program()
    res = run_bass_kernel_spmd(nc, in_maps, list(range(NCORES)))
    return np.asarray(res.results[0]["out"], dtype=np.float32)


# revision 16
# speedup vs baseline: 2.1702x; 1.3405x over previous
"""GTN (graph transformer network) meta-path kernel for TRN2, 8 NeuronCores.

Math (reference):
    Ap = A transposed to [E, N, N]
    a  = sum_e softmax(w1_0)[c,e] * Ap[e]      (per channel c)
    b  = sum_e softmax(w2_0)[c,e] * Ap[e]
    H  = a @ b
    twice:  H = normalize(H) @ gtconv(Ap, w)   (normalize = zero diag, col-scale)
    out = symmetrized mean over channels.

Sharding: channel-parallel — core c computes channel c end to end (the four
softmax mixes differ only in the tiny [E] weight vector, fed per-core), then
one AllReduce over the 8 cores and a local symmetrization.

On-device formulation works with TRANSPOSED intermediates Ht = H^T so that
 - every GEMM's moving operand is the previous GEMM's output as-is,
 - normalization becomes row sums (free-dim reduce) + per-partition scale.

v2 design notes (vs the packed/unpack baseline):
 - A is fed once in a host-permuted bf16 layout At3[b, (k16 e), j]; one PE
   pass with a block-diagonal [128, 64] weight computes all four mixes.
 - Mix outputs are written DIRECTLY to a natural-layout fp8 tensor nat4
   [4, N, N] (no DRAM packed round-trip): full-row staging in SBUF gives
   per-h scatter DMAs with 3-dim APs (q, k, j) and 2KB contiguous lines.
 - A-loads round-robin across the three DMA-capable queues (sync / scalar /
   gpsimd) — a single queue tops out ~130-185 GB/s and serialized the
   baseline's mix phase.
 - GEMMs run in fp8 (e4m3) with DoubleRow perf mode (contract 256/instr).
   Normalized intermediates are scaled by N so fp8 sees ~1.0-magnitudes
   (normalize is scale-invariant, so the chain stays exact); the single
   1/(N*16) correction is folded into the GEMM3 PSUM eviction.
 - h2t / AllReduce in bf16 (halves collective + tail HBM traffic).
"""

import numpy as np

N = 2048
E = 8
C = 8
P = 128
NCORES = 8
SCALE_N = float(N)          # fp8 re-scale applied at each normalize
EVICT_SCALE = 1.0 / (SCALE_N * 16.0)  # undo SCALE_N; /8 channel mean; /2 symm

_PROGRAM = None


def _softmax_rows(w: np.ndarray) -> np.ndarray:
    """w: [C, E, 1, 1] -> softmax over E, float64 precision, returns [C, E]."""
    x = w.reshape(C, E).astype(np.float64)
    x = x - x.max(axis=1, keepdims=True)
    ex = np.exp(x)
    return ex / ex.sum(axis=1, keepdims=True)


def _build_program():
    import concourse.bacc as bacc
    import concourse.mybir as mybir
    import concourse.tile as tile
    from concourse.masks import make_identity

    f32 = mybir.dt.float32
    bf16 = mybir.dt.bfloat16
    f8 = mybir.dt.float8e4
    AX = mybir.AxisListType.X
    MUL = mybir.AluOpType.mult
    ADD = mybir.AluOpType.add
    NE = mybir.AluOpType.not_equal
    COPY = mybir.ActivationFunctionType.Copy
    DR = mybir.MatmulPerfMode.DoubleRow

    nc = bacc.Bacc("TRN2")
    # A ships as fp8 (mixed-dtype matmul: bf16 stationary x fp8 moving is
    # supported by the PE and verified bit-accurate on HW) — halves the
    # dominant phase-1 HBM read.
    A3_ext = nc.dram_tensor("At3", [P, P, N], f8, kind="ExternalInput")
    w4_ext = nc.dram_tensor("wblk4", [P, 64], bf16, kind="ExternalInput")
    # each core emits only its 256-row band; the host concatenates
    out_ext = nc.dram_tensor("out", [256, N], f32, kind="ExternalOutput")

    with tile.TileContext(nc) as tc:
        with (
            tc.tile_pool(name="dram", bufs=1, space="DRAM") as dpool,
            tc.tile_pool(name="const", bufs=1) as cpool,
        ):
            # all four mixes in natural layout, fp8: nat4[q, i, j]
            nat4 = dpool.tile([4, N, N], f8, name="nat4")
            # per-channel H''^T (bf16): row-major copy for the row-band
            # ReduceScatter, plus a column-banded copy X[b][c, i, j] =
            # T[512b+i, 256c+j] in 4 band tiles so the column ReduceScatter
            # pipelines with GEMM3.
            h2t = dpool.tile([N, N], bf16, name="h2t")
            xb = [
                dpool.tile([8, 512, 256], bf16, name=f"xb{b}") for b in range(4)
            ]
            # RS results: r1 = S[band_c, :], r2 = S[:, band_c] (this core's c)
            # (ReduceScatter outputs are per-rank local, not Shared)
            r1 = dpool.tile([256, N], bf16, name="r1")
            r2 = dpool.tile([N, 256], bf16, name="r2")

            # --- constants ---
            w4_sb = cpool.tile([P, 64], bf16)
            nc.sync.dma_start(out=w4_sb[:], in_=w4_ext[:])
            identb = cpool.tile([P, P], bf16)
            make_identity(nc, identb[:])
            identf8 = cpool.tile([P, P], f8)
            make_identity(nc, identf8[:])
            # diag masks: masks[:, v, y] = 0 where y == p + v*128 else 1
            masks = cpool.tile([P, 4, 512], f32)
            nc.gpsimd.memset(masks[:], 1.0)
            for v in range(4):
                nc.gpsimd.affine_select(
                    out=masks[:, v],
                    in_=masks[:, v],
                    compare_op=NE,
                    fill=0.0,
                    base=v * P,
                    pattern=[[-1, 512]],
                    channel_multiplier=1,
                )

            # natural-layout scatter view of nat4 for the mix writes:
            # natural row i = 32*bp + 16*h + k ; partition p of a staged mix
            # tile is (h, q, k) -> per-h destination AP dims (q, k, j): 3-dim.
            nat_sc = nat4[:].rearrange(
                "q (bp h k) j -> bp h q k j", h=2, k=16
            )

            # =========== Phase 1: all four mixes in one PE pass ===========
            # direction-segregated queues: reads on sync+scalar (HWDGE),
            # scatter writes on gpsimd (software DGE) — mixing loads and
            # scattered stores on one HWDGE queue measured ~45 GB/s.
            ld_engs = [nc.sync, nc.scalar]
            with (
                tc.tile_pool(name="mix", bufs=3) as mpool,
                tc.tile_pool(name="mixst", bufs=3) as spool,
                tc.tile_pool(name="mixps", bufs=6, space="PSUM") as mpsum,
            ):
                for ld4 in range(32):
                    a3t = mpool.tile([P, 4, N], f8, tag="a3t")
                    ld_engs[ld4 % 2].dma_start(
                        out=a3t[:],
                        in_=A3_ext[4 * ld4 : 4 * ld4 + 4].rearrange(
                            "b p j -> p b j"
                        ),
                    )
                    for half in range(2):
                        bp = ld4 * 2 + half
                        # full-row staging: st holds both h's for this bp
                        st = spool.tile([P, N], f8, tag="st")
                        for jc in range(4):
                            pm = mpsum.tile([P, 512], f32, tag="pm")
                            for h in range(2):
                                nc.tensor.matmul(
                                    pm[h * 64 : (h + 1) * 64, :],
                                    lhsT=w4_sb[:],
                                    rhs=a3t[
                                        :,
                                        half * 2 + h,
                                        jc * 512 : (jc + 1) * 512,
                                    ],
                                    start=True,
                                    stop=True,
                                )
                            if jc % 2 == 0:
                                nc.vector.tensor_copy(
                                    out=st[:, jc * 512 : (jc + 1) * 512],
                                    in_=pm[:],
                                )
                            else:
                                nc.scalar.copy(
                                    st[:, jc * 512 : (jc + 1) * 512], pm[:]
                                )
                        # one scatter DMA per h: dest dims (q, k, j=2048)
                        nc.gpsimd.dma_start(
                            out=nat_sc[bp, 0], in_=st[0:64, :]
                        )
                        nc.gpsimd.dma_start(
                            out=nat_sc[bp, 1], in_=st[64:128, :]
                        )

            # =========== Phases 2-4: three chained GEMMs (fp8) ===========
            with (
                tc.tile_pool(name="big", bufs=1) as bigpool,
                tc.tile_pool(name="gw", bufs=3) as gpool,
                tc.tile_pool(name="nrm", bufs=4) as npool,
                tc.tile_pool(name="gps", bufs=2, space="PSUM") as gpsum,
            ):
                mv = [
                    bigpool.tile([P, 16, N], f8, tag="mv0", name="mva"),
                    bigpool.tile([P, 16, N], f8, tag="mv1", name="mvb"),
                ]

                # Build mv0 = a^T chunks by PE-transposing nat4[0] blocks
                anat_v = nat4[0].rearrange("(ib p) k -> p ib k", p=P)
                for kc in range(16):
                    ld = gpool.tile([P, 16, P], f8, tag="ld")
                    nc.sync.dma_start(
                        out=ld[:], in_=anat_v[:, :, kc * P : (kc + 1) * P]
                    )
                    for ib4 in range(4):
                        # fp8 transpose mode requires output element step 2:
                        # allocate 2x-wide psum and write via stride-2 view
                        tp = gpsum.tile(
                            [P, 1024], f8, tag=f"ps{ib4 % 2}", name="tp"
                        )
                        tpv = tp[:].rearrange("p (x two) -> p two x", two=2)[
                            :, 0
                        ]
                        for g in range(4):
                            nc.tensor.transpose(
                                tpv[:, g * P : (g + 1) * P],
                                ld[:, ib4 * 4 + g, :],
                                identf8[:],
                            )
                        if ib4 % 2 == 0:
                            nc.vector.tensor_copy(
                                out=mv[0][:, kc, ib4 * 512 : (ib4 + 1) * 512],
                                in_=tpv[:],
                            )
                        else:
                            nc.scalar.copy(
                                mv[0][:, kc, ib4 * 512 : (ib4 + 1) * 512],
                                tpv[:],
                            )

                def gemm(qi, rhs_res, out_res, normalize):
                    """Transposed-chain GEMM: out = mix_q^T @ rhs (DoubleRow).

                    qi: q index in nat4 (1=b, 2=g1, 3=g2).
                    rhs_res: SBUF-resident moving operand [P, 16, N] fp8.
                    out_res: SBUF [P, 16, N] fp8 (normalize, scaled by N) or
                        None (evict bf16 to h2t with EVICT_SCALE).
                    """
                    natq = nat4[qi].rearrange("(kc p) j -> p kc j", p=P)
                    for ms2 in range(8):
                        # stationary for two ms blocks: 512B lines
                        bts = gpool.tile([P, 16, 2 * P], f8, tag="bts")
                        nc.sync.dma_start(
                            out=bts[:],
                            in_=natq[:, :, ms2 * 2 * P : (ms2 + 1) * 2 * P],
                        )
                        for msh in range(2):
                            ms = ms2 * 2 + msh
                            ps = [
                                gpsum.tile(
                                    [P, 512], f32, tag=f"ps{ic}", name=f"ps{ic}"
                                )
                                for ic in range(4)
                            ]
                            for kc2 in range(8):
                                for ic in range(4):
                                    nc.tensor.matmul(
                                        ps[ic][:],
                                        lhsT=bts[
                                            :,
                                            2 * kc2 : 2 * kc2 + 2,
                                            msh * P : (msh + 1) * P,
                                        ],
                                        rhs=rhs_res[
                                            :,
                                            2 * kc2 : 2 * kc2 + 2,
                                            ic * 512 : (ic + 1) * 512,
                                        ],
                                        start=(kc2 == 0),
                                        stop=(kc2 == 7),
                                        perf_mode=DR,
                                    )
                            if normalize:
                                dc = (ms * P) // 512
                                v = ms % 4
                                degp = npool.tile([P, 4], f32, tag="degp")
                                # zero diagonal in place + row-sum of masked tile
                                nc.vector.scalar_tensor_tensor(
                                    out=ps[dc][:],
                                    in0=ps[dc][:],
                                    scalar=1.0,
                                    in1=masks[:, v],
                                    op0=MUL,
                                    op1=MUL,
                                    accum_out=degp[:, dc : dc + 1],
                                )
                                for ic in range(4):
                                    if ic != dc:
                                        nc.vector.tensor_reduce(
                                            degp[:, ic : ic + 1], ps[ic][:],
                                            AX, ADD,
                                        )
                                degs = npool.tile([P, 1], f32, tag="degs")
                                nc.vector.tensor_reduce(degs[:], degp[:], AX, ADD)
                                dinv = npool.tile([P, 1], f32, tag="dinv")
                                nc.vector.reciprocal(dinv[:], degs[:])
                                # fp8 range: scale normalized rows by N (~1.0)
                                nc.vector.tensor_scalar_mul(
                                    dinv[:], dinv[:], SCALE_N
                                )
                                for ic in range(4):
                                    nc.scalar.activation(
                                        out_res[:, ms, ic * 512 : (ic + 1) * 512],
                                        ps[ic][:],
                                        COPY,
                                        scale=dinv[:],
                                    )
                            else:
                                xv = xb[ms // 4][:].rearrange("c i j -> i c j")
                                for ic in range(4):
                                    st = gpool.tile([P, 512], bf16, tag="fstage")
                                    nc.scalar.activation(
                                        st[:], ps[ic][:], COPY, scale=EVICT_SCALE
                                    )
                                    nc.scalar.dma_start(
                                        out=h2t[
                                            ms * P : (ms + 1) * P,
                                            ic * 512 : (ic + 1) * 512,
                                        ],
                                        in_=st[:],
                                    )
                                    # column-banded copy for the col-band RS
                                    nc.gpsimd.dma_start(
                                        out=xv[
                                            (ms % 4) * P : (ms % 4 + 1) * P,
                                            2 * ic : 2 * ic + 2,
                                            :,
                                        ],
                                        in_=st[:],
                                    )

                # GEMM1: Ht = b^T a^T ; normalize -> N*Hnt in mv[1]
                gemm(1, mv[0], mv[1], normalize=True)
                # GEMM2: H't = g1^T Hnt ; normalize -> N*H'nt (reuse mv0 slot)
                mv0b = bigpool.tile([P, 16, N], f8, tag="mv0")
                gemm(2, mv[1], mv0b, normalize=True)
                # GEMM3: H''t = g2^T H'nt -> h2t (bf16, EVICT_SCALE folded)
                gemm(3, mv0b, None, normalize=False)

                # ===== Phase 5: two ReduceScatters =====
                # Col-band RS: core c receives S[:, 256c:256(c+1)] in 4 row
                # bands; each band's input tile completes mid-GEMM3, so these
                # pipeline with the remaining GEMM3 compute.
                for b in range(4):
                    nc.gpsimd.collective_compute(
                        "ReduceScatter",
                        ADD,
                        replica_groups=[list(range(NCORES))],
                        ins=[xb[b].opt()],
                        outs=[r2[512 * b : 512 * (b + 1), :].opt()],
                    )
                # Row-band RS: core c receives S[256c:256(c+1), :].
                nc.gpsimd.collective_compute(
                    "ReduceScatter",
                    ADD,
                    replica_groups=[list(range(NCORES))],
                    ins=[h2t[:].opt()],
                    outs=[r1[:].opt()],
                )

                # ===== Phase 6: out band = r1 + r2^T  (256 rows/core) =====
                r2sb = gpool.tile([P, 16, 256], bf16, tag="r2sb", bufs=1)
                nc.sync.dma_start(
                    out=r2sb[:],
                    in_=r2[:].rearrange("(ib p) j -> p ib j", p=P),
                )
                for mso in range(2):
                    srow = gpool.tile([P, N], bf16, tag="srow", bufs=2)
                    nc.scalar.dma_start(
                        out=srow[:], in_=r1[mso * P : (mso + 1) * P, :]
                    )
                    ost = gpool.tile([P, N], f32, tag="ost", bufs=2)
                    for jb4 in range(4):
                        pst = gpsum.tile(
                            [P, 512], bf16, tag=f"ps{jb4 % 2}", name="pst"
                        )
                        for g in range(4):
                            nc.tensor.transpose(
                                pst[:, g * P : (g + 1) * P],
                                r2sb[:, jb4 * 4 + g, mso * P : (mso + 1) * P],
                                identb[:],
                            )
                        nc.vector.scalar_tensor_tensor(
                            out=ost[:, jb4 * 512 : (jb4 + 1) * 512],
                            in0=srow[:, jb4 * 512 : (jb4 + 1) * 512],
                            scalar=1.0,
                            in1=pst[:],
                            op0=MUL,
                            op1=ADD,
                        )
                    (nc.sync if mso == 0 else nc.scalar).dma_start(
                        out=out_ext[mso * P : (mso + 1) * P, :], in_=ost[:]
                    )


    nc.compile()
    return nc


def _get_program():
    global _PROGRAM
    if _PROGRAM is None:
        _PROGRAM = _build_program()
    return _PROGRAM


def _make_wblk(sws) -> np.ndarray:
    """Block-diagonal mix weights [128, 16*len(sws)].

    wblk[(x*8+e), (q*16+x)] = sws[q][e]  for x in 0..15.
    Partitions = (16 x, 8 e) matching the host-permuted A layout; out
    partitions = (q, 16 x).
    """
    wblk = np.zeros((P, 16 * len(sws)), np.float32)
    for q, sw in enumerate(sws):
        for x in range(16):
            wblk[x * 8 : (x + 1) * 8, q * 16 + x] = sw.astype(np.float32)
    return wblk


def _prep_inputs(A, w1_0, w2_0, w_1, w_2):
    import ml_dtypes

    swa = _softmax_rows(np.asarray(w1_0))
    swb = _softmax_rows(np.asarray(w2_0))
    sg1 = _softmax_rows(np.asarray(w_1))
    # channel-mean/symmetrize scale folded into GEMM3's PSUM eviction
    sg2 = _softmax_rows(np.asarray(w_2))

    af8 = np.asarray(A, dtype=np.float32)[0].astype(
        ml_dtypes.float8_e4m3fn
    )  # [k,j,e]; values in [0,1) — identical encoding to TRN e4m3 there
    # At3[b, (k16 e), j] = A[16b+k16, j, e]
    at3 = np.ascontiguousarray(af8.transpose(0, 2, 1).reshape(P, P, N))
    in_maps = []
    for c in range(NCORES):
        w4 = _make_wblk([swa[c], swb[c], sg1[c], sg2[c]]).astype(
            ml_dtypes.bfloat16
        )
        in_maps.append({"At3": at3, "wblk4": w4})
    return in_maps


def kernel(A, w1_0, w2_0, w_1, w_2):
    from concourse.bass_utils import run_bass_kernel_spmd

    in_maps = _prep_inputs(A, w1_0, w2_0, w_1, w_2)
    nc = _get_program()
    res = run_bass_kernel_spmd(nc, in_maps, list(range(NCORES)))
    return np.concatenate(
        [np.asarray(res.results[c]["out"], dtype=np.float32) for c in range(NCORES)],
        axis=0,
    )
